# revision 55
# baseline (speedup 1.0000x reference)
"""Multi-Head Latent Attention (MLA) Trainium2 kernel, 8-core SPMD.

Latent sharding: core c -> batch b = c // 4, S-chunk g = c % 4 (512 rows).
SDPA/decompress sharding: core c -> global heads {2c, 2c+1} for BOTH
batches (4 head-instances per core) so that every collective is a single
8-wide op on [[0..7]] (4-wide AllToAll is unsupported on trn2 mesh).

Flow:
 - Phase A: latent path (cq / ckv / k_rope_raw + rmsnorm) on own chunk;
   kv latent (+rotated krope) AllGathered 8-wide early.
 - Phase B: decompress q_nope and (rotated) q_rope for ALL 16 heads on own
   chunk (same total FLOPs as 4-head x full-S), then ONE 8-wide AllToAll
   delivers each head-pair to its owner.  No serial AG->decompress stall.
 - Phase C: decompress k/v for own 2 heads x both batches from gathered
   latent; assemble qT/kT.
 - Phase D: causal SDPA per (head, batch), scores transposed ([sk, sq]);
   denominators via ones-matmul.  Attention outputs leave through one
   8-wide AllToAll per head-slot (block = (batch, sq-window)), pipelined.
 - Phase E: after the A2As each core holds all 16 heads restricted to its
   own (batch, 512-row chunk); projection is S-sharded with full w_proj.

Layout notes:
 - All matmul operands bf16 (fp32 PSUM accumulate); rope pair-dims are
   permuted (even dims first) so rotation works on contiguous 32-blocks,
   applied identically to q and k so dot products are unchanged.
 - Odd heads use a half-swapped partition layout ([rope | nope]) in both
   qT and kT so every PSUM eviction is partition-aligned.
 - q_norm_w / kv_norm_w are folded into the decompress weights on host.
 - q A2A block j (256 rows): per head slot (2): [0:64] q_nope (-> qT),
   [64:128] rotated per-head q_rope (-> kT rope slot, per the reference's
   rope swap).
"""

import sys

for _p in ("/opt/trn_rl_repo", "/opt/pypackages"):
    if _p not in sys.path:
        sys.path.append(_p)

import numpy as np
import ml_dtypes

B, S, D = 2, 2048, 2048
H, HD, RD, ND = 16, 128, 64, 64
QR, KVR = 1536, 512
EPS = 1e-6
G = 4            # S-chunks per batch
NC = 8
SC = S // G      # latent-path S chunk per core
NT = S // 128    # 16 s-tiles
NW = S // 512    # 4  sq windows
LATW = QR + KVR + RD   # 2112 = packed cq|ckv|krope width
SCALE = 1.0 / float(np.sqrt(HD))
NEG = -30000.0   # additive mask; * SCALE stays << exp underflow

BF = ml_dtypes.bfloat16

_cached = {}


def _build():
    import concourse.bass as bass
    import concourse.mybir as mybir
    import concourse.tile as tile
    from concourse import bacc
    from concourse.masks import make_identity
    from contextlib import ExitStack

    f32 = mybir.dt.float32
    bf16 = mybir.dt.bfloat16
    f8 = mybir.dt.float8e4
    DR = mybir.MatmulPerfMode.DoubleRow
    AF = mybir.ActivationFunctionType

    nc = bacc.Bacc()

    # ---- parameters (per-core host-prepped) ----
    P_xT = nc.declare_dram_parameter("xT", [D, SC], bf16, isOutput=False)
    P_wlat = nc.declare_dram_parameter("wlat", [D, LATW], bf16, isOutput=False)
    P_wdqn = nc.declare_dram_parameter("wdqn", [QR, H * ND], bf16, isOutput=False)
    P_wdqr = nc.declare_dram_parameter("wdqr", [QR, H * RD], bf16, isOutput=False)
    P_wdkn = nc.declare_dram_parameter("wdkn", [KVR, 2 * ND], bf16, isOutput=False)
    P_wdv = nc.declare_dram_parameter("wdv", [KVR, 2 * HD], bf16, isOutput=False)
    P_wproj = nc.declare_dram_parameter("wproj", [H * HD, H * HD], bf16, isOutput=False)
    # rope tables (own chunk), A variant = [cos|sin], B = [sin|cos]
    P_csAc = nc.declare_dram_parameter("csAc", [SC, RD], bf16, isOutput=False)
    P_csBc = nc.declare_dram_parameter("csBc", [SC, RD], bf16, isOutput=False)
    P_csAc8 = nc.declare_dram_parameter("csAc8", [SC, 8 * RD], bf16, isOutput=False)
    P_csBc8 = nc.declare_dram_parameter("csBc8", [SC, 8 * RD], bf16, isOutput=False)
    P_mask = nc.declare_dram_parameter("maskT", [128, 128], f32, isOutput=False)
    P_out = nc.declare_dram_parameter("out", [SC, H * HD], f32, isOutput=True)

    groups = [[0, 1, 2, 3, 4, 5, 6, 7]]

    with ExitStack() as top:
        tc = top.enter_context(tile.TileContext(nc))

        dram = top.enter_context(tc.tile_pool(name="dram", bufs=1, space="DRAM"))
        KVW = KVR + RD   # 576
        gkv_in = dram.tile([KVW, SC], bf16, tag="gkv_in", name="gkv_in")
        gkv_out = dram.tile([NC, KVW, SC], bf16, tag="gkv_out", name="gkv_out")
        # q A2A: block j = heads {2j, 2j+1} for my S chunk; 128 rows/head
        gqa_in = dram.tile([NC, 256, SC], bf16, tag="gqa_in", name="gqa_in")
        gqa_out = dram.tile([NC, 256, SC], bf16, tag="gqa_out", name="gqa_out")
        # attn-out A2A per local head slot: block j = (batch j//4, window j%4)
        ago_in = [dram.tile([NC, HD, 512], bf16, tag=f"ago_in{h}", name=f"ago_in{h}")
                  for h in range(2)]
        ago_out = [dram.tile([NC, HD, 512], bf16, tag=f"ago_out{h}", name=f"ago_out{h}")
                   for h in range(2)]

        const = top.enter_context(tc.tile_pool(name="const", bufs=1))
        ident = const.tile([128, 128], bf16, tag="ident", name="ident")
        make_identity(nc, ident)
        ones_sb = const.tile([128, 128], bf16, tag="ones", name="ones")
        nc.vector.memset(ones_sb[:], 1.0)
        mask_sb = const.tile([128, 128], f32, tag="mask", name="mask")
        nc.sync.dma_start(mask_sb[:], P_mask[:])
        eps_sb = const.tile([128, 1], f32, tag="eps", name="eps")
        nc.vector.memset(eps_sb[:], EPS)

        with ExitStack() as ctxAB:
            # transposed normed q latent, persists from Phase A into Phase B
            nqp = ctxAB.enter_context(tc.tile_pool(name="nqp", bufs=1))
            nqTl = [nqp.tile([128, SC], bf16, tag=f"nqT{rt}", name=f"nqT{rt}")
                    for rt in range(QR // 128)]
            # q decompress weights + tables (used inside pass 2); DMAs are
            # issued after pass 1 so they don't crowd pass 1's inputs
            wdqr_sb = [nqp.tile([128, H * RD], bf16, tag=f"wdqr{rt}",
                                name=f"wdqr{rt}") for rt in range(QR // 128)]
            wdqn_sb = [nqp.tile([128, H * ND], bf16, tag=f"wdqn{rt}",
                                name=f"wdqn{rt}") for rt in range(QR // 128)]
            csA8_sb = [nqp.tile([128, 512], bf16, tag=f"csA8{st}",
                                name=f"csA8{st}") for st in range(SC // 128)]
            csB8_sb = [nqp.tile([128, 512], bf16, tag=f"csB8{st}",
                                name=f"csB8{st}") for st in range(SC // 128)]

            # ================= Phase A: latent path on own S chunk =============
            with ExitStack() as ctxA:
                pa = ctxA.enter_context(tc.tile_pool(name="pa", bufs=1))
                pa_mv = ctxA.enter_context(tc.tile_pool(name="pa_mv", bufs=3))
                pa_ps = ctxA.enter_context(
                    tc.tile_pool(name="pa_ps", bufs=6, space="PSUM"))
                pa_tp = ctxA.enter_context(
                    tc.tile_pool(name="pa_tp", bufs=2, space="PSUM"))

                xT_sb = []
                wlat_sb = []
                for dt_ in range(D // 128):
                    xt = pa.tile([128, SC], bf16, tag=f"xT{dt_}", name=f"xT{dt_}")
                    nc.sync.dma_start(xt[:], P_xT[dt_ * 128:(dt_ + 1) * 128, :])
                    xT_sb.append(xt)
                    wl = pa.tile([128, LATW], bf16, tag=f"wlat{dt_}", name=f"wlat{dt_}")
                    nc.sync.dma_start(wl[:, 1536:LATW],
                                      P_wlat[dt_ * 128:(dt_ + 1) * 128, 1536:LATW])
                    wlat_sb.append(wl)
                for dt_ in range(D // 128):
                    nc.sync.dma_start(wlat_sb[dt_][:, 0:1536],
                                      P_wlat[dt_ * 128:(dt_ + 1) * 128, 0:1536])
                csAc_sb, csBc_sb = [], []
                for st in range(SC // 128):
                    t = pa.tile([128, RD], bf16, tag=f"csAc{st}", name=f"csAc{st}")
                    nc.sync.dma_start(t[:], P_csAc[st * 128:(st + 1) * 128, :])
                    csAc_sb.append(t)
                    t = pa.tile([128, RD], bf16, tag=f"csBc{st}", name=f"csBc{st}")
                    nc.sync.dma_start(t[:], P_csBc[st * 128:(st + 1) * 128, :])
                    csBc_sb.append(t)

                # ---- PASS 1: kv + krope columns only, so their AllGather
                # fires early and absorbs cross-rank skew ----
                for st in range(SC // 128):
                    pkv = pa_ps.tile([128, 512], f32, tag="lat_ps", name="lat_ps")
                    pkr = pa_ps.tile([128, RD], f32, tag="lat_ps", name="lat_ps")
                    for dt_ in range(D // 128):
                        stat = xT_sb[dt_][:, st * 128:(st + 1) * 128]
                        first, last = dt_ == 0, dt_ == D // 128 - 1
                        nc.tensor.matmul(
                            pkv[:], stat, wlat_sb[dt_][:, 1536:2048],
                            start=first, stop=last)
                        nc.tensor.matmul(
                            pkr[:], stat, wlat_sb[dt_][:, 2048:LATW],
                            start=first, stop=last)
                    kvn_sb = pa_mv.tile([128, KVW], bf16, tag="kvn_sb", name="kvn_sb")
                    acckv = pa_mv.tile([128, 1], f32, tag="acckv", name="acckv")
                    sqkv = pa_mv.tile([128, 512], f32, tag="sqkv", name="sqkv")
                    nc.scalar.activation(sqkv[:], pkv[:], AF.Square,
                                         accum_out=acckv[:])
                    stdkv = pa_mv.tile([128, 1], f32, tag="stdkv", name="stdkv")
                    nc.scalar.activation(stdkv[:], acckv[:], AF.Sqrt,
                                         bias=eps_sb[:], scale=1.0 / KVR)
                    rkv = pa_mv.tile([128, 1], f32, tag="rkv", name="rkv")
                    nc.vector.reciprocal(rkv[:], stdkv[:])
                    nc.vector.tensor_scalar_mul(kvn_sb[:, 0:512], pkv[:], rkv[:])
                    # krope: rotate (no norm)
                    kr_raw = pa_mv.tile([128, RD], bf16, tag="kr_raw", name="kr_raw")
                    nc.scalar.copy(kr_raw[:], pkr[:])
                    pr1 = pa_mv.tile([128, RD], bf16, tag="pr1", name="pr1")
                    pr2 = pa_mv.tile([128, RD], bf16, tag="pr2", name="pr2")
                    nc.vector.tensor_mul(pr1[:], kr_raw[:], csAc_sb[st][:])
                    nc.vector.tensor_mul(pr2[:], kr_raw[:], csBc_sb[st][:])
                    nc.vector.tensor_sub(kvn_sb[:, 512:544],
                                         pr1[:, 0:32], pr1[:, 32:64])
                    nc.vector.tensor_add(kvn_sb[:, 544:576],
                                         pr2[:, 0:32], pr2[:, 32:64])
                    for rt in range(4):
                        tp = pa_tp.tile([128, 128], bf16, tag="tp", name="tp")
                        nc.tensor.transpose(
                            tp[:], kvn_sb[:, rt * 128:(rt + 1) * 128], ident[:])
                        tps = pa_mv.tile([128, 128], bf16, tag="tps", name="tps")
                        nc.scalar.copy(tps[:], tp[:])
                        nc.scalar.dma_start(
                            gkv_in[rt * 128:(rt + 1) * 128,
                                   st * 128:(st + 1) * 128], tps[:])
                    tp = pa_tp.tile([128, 128], bf16, tag="tp", name="tp")
                    nc.tensor.transpose(tp[0:64, :], kvn_sb[:, 512:576], ident[:])
                    tps = pa_mv.tile([128, 128], bf16, tag="tps", name="tps")
                    nc.scalar.copy(tps[0:64, :], tp[0:64, :])
                    nc.scalar.dma_start(
                        gkv_in[KVR:KVW, st * 128:(st + 1) * 128], tps[0:64, :])

                nc.gpsimd.collective_compute(
                    "AllGather", mybir.AluOpType.bypass,
                    replica_groups=groups,
                    ins=[gkv_in.opt()], outs=[gkv_out.opt()])

                for rt in range(QR // 128):
                    nc.sync.dma_start(wdqr_sb[rt][:],
                                      P_wdqr[rt * 128:(rt + 1) * 128, :])
                for st in range(SC // 128):
                    nc.sync.dma_start(csA8_sb[st][:],
                                      P_csAc8[st * 128:(st + 1) * 128, :])
                    nc.sync.dma_start(csB8_sb[st][:],
                                      P_csBc8[st * 128:(st + 1) * 128, :])
                for rt in range(QR // 128):
                    nc.sync.dma_start(wdqn_sb[rt][:],
                                      P_wdqn[rt * 128:(rt + 1) * 128, :])

                # ---- PASS 2: q columns (cq + rmsnorm), transposed into SBUF ----
                for st in range(SC // 128):
                    ps = []
                    for j in range(3):
                        p = pa_ps.tile([128, 512], f32, tag="lat_ps", name="lat_ps")
                        ps.append(p)
                    for dt_ in range(D // 128):
                        stat = xT_sb[dt_][:, st * 128:(st + 1) * 128]
                        first, last = dt_ == 0, dt_ == D // 128 - 1
                        for j in range(3):
                            nc.tensor.matmul(
                                ps[j][:], stat,
                                wlat_sb[dt_][:, j * 512:(j + 1) * 512],
                                start=first, stop=last)
                    norm_sb = pa_mv.tile([128, QR], bf16, tag="norm_sb", name="norm_sb")
                    acc = [pa_mv.tile([128, 1], f32, tag=f"acc{i}", name=f"acc{i}")
                           for i in range(3)]
                    for i in range(3):
                        sq = pa_mv.tile([128, 512], f32, tag=f"sq{i}", name=f"sq{i}")
                        nc.scalar.activation(sq[:], ps[i][:], AF.Square,
                                             accum_out=acc[i][:])
                    accq = pa_mv.tile([128, 1], f32, tag="accq", name="accq")
                    nc.vector.tensor_add(accq[:], acc[0][:], acc[1][:])
                    nc.vector.tensor_add(accq[:], accq[:], acc[2][:])
                    stdq = pa_mv.tile([128, 1], f32, tag="stdq", name="stdq")
                    nc.scalar.activation(stdq[:], accq[:], AF.Sqrt,
                                         bias=eps_sb[:], scale=1.0 / QR)
                    rq = pa_mv.tile([128, 1], f32, tag="rq", name="rq")
                    nc.vector.reciprocal(rq[:], stdq[:])
                    for j in range(3):
                        nc.vector.tensor_scalar_mul(
                            norm_sb[:, j * 512:(j + 1) * 512], ps[j][:], rq[:])
                    for rt in range(12):
                        tp = pa_tp.tile([128, 128], bf16, tag="tp", name="tp")
                        nc.tensor.transpose(
                            tp[:], norm_sb[:, rt * 128:(rt + 1) * 128], ident[:])
                        nc.scalar.copy(
                            nqTl[rt][:, st * 128:(st + 1) * 128], tp[:])

                    # ---- q_rope for this st: decompress (natural), rotate,
                    # transpose into A2A slots -- pipelined behind pass 2 ----
                    pn = [pa_ps.tile([128, 512], f32, tag="lat_ps", name="lat_ps")
                          for _ in range(2)]
                    for rt in range(QR // 128):
                        stat = nqTl[rt][:, st * 128:(st + 1) * 128]
                        for j in range(2):
                            nc.tensor.matmul(
                                pn[j][:], stat,
                                wdqr_sb[rt][:, j * 512:(j + 1) * 512],
                                start=rt == 0, stop=rt == QR // 128 - 1)
                    for j in range(2):
                        pr1 = pa_mv.tile([128, 512], bf16, tag="qpr1", name="qpr1")
                        pr2 = pa_mv.tile([128, 512], bf16, tag="qpr2", name="qpr2")
                        nc.vector.tensor_mul(pr1[:], pn[j][:], csA8_sb[st][:])
                        nc.vector.tensor_mul(pr2[:], pn[j][:], csB8_sb[st][:])
                        rot = pa_mv.tile([128, 512], bf16, tag="qrot", name="qrot")
                        r3a = rot[:].rearrange("p (h two f) -> p h two f", two=2, f=32)
                        p3a = pr1[:].rearrange("p (h two f) -> p h two f", two=2, f=32)
                        p3b = pr2[:].rearrange("p (h two f) -> p h two f", two=2, f=32)
                        nc.vector.tensor_sub(r3a[:, :, 0, :], p3a[:, :, 0, :],
                                             p3a[:, :, 1, :])
                        nc.vector.tensor_add(r3a[:, :, 1, :], p3b[:, :, 0, :],
                                             p3b[:, :, 1, :])
                        for pr in range(4):   # head pair within this 8-head tile
                            blk = 4 * j + pr
                            tp = pa_tp.tile([128, 128], bf16, tag="tp", name="tp")
                            nc.tensor.transpose(
                                tp[:], rot[:, pr * 128:(pr + 1) * 128], ident[:])
                            tps = pa_mv.tile([128, 128], bf16, tag="qtps", name="qtps")
                            nc.scalar.copy(tps[:], tp[:])
                            eng = (nc.scalar, nc.gpsimd, nc.scalar, nc.gpsimd)[pr]
                            eng.dma_start(
                                gqa_in[blk, 64:128, st * 128:(st + 1) * 128],
                                tps[0:64, :])
                            eng.dma_start(
                                gqa_in[blk, 192:256, st * 128:(st + 1) * 128],
                                tps[64:128, :])

            # ======= Phase B: q_nope decompress (transposed form) =======
            with ExitStack() as ctxB:
                pb_mv = ctxB.enter_context(tc.tile_pool(name="pb_mv", bufs=4))
                pb_ps = ctxB.enter_context(
                    tc.tile_pool(name="pb_ps", bufs=3, space="PSUM"))
                # psum tile p = A2A block p (heads 2p, 2p+1)
                for p in range(H // 2):
                    ps = pb_ps.tile([128, 512], f32, tag="qn_ps", name="qn_ps")
                    for rt in range(QR // 128):
                        nc.tensor.matmul(
                            ps[:], wdqn_sb[rt][:, p * 128:(p + 1) * 128],
                            nqTl[rt][:],
                            start=rt == 0, stop=rt == QR // 128 - 1)
                    qn_sb = pb_mv.tile([128, 512], bf16, tag="qn_sb", name="qn_sb")
                    nc.scalar.copy(qn_sb[:], ps[:])
                    nc.scalar.dma_start(gqa_in[p, 0:64, :], qn_sb[0:64, :])
                    nc.scalar.dma_start(gqa_in[p, 128:192, :], qn_sb[64:128, :])

            nc.gpsimd.collective_compute(
                "AllToAll", mybir.AluOpType.bypass,
                replica_groups=groups,
                ins=[gqa_in.opt()], outs=[gqa_out.opt()])

        # ================= Phase C: decompress k/v, assemble q =================
        persist = top.enter_context(tc.tile_pool(name="persist", bufs=1))
        wpj = []
        for ot in range(H * HD // 128):
            t = persist.tile([128, H * HD], bf16, tag=f"wpj{ot}", name=f"wpj{ot}")
            nc.sync.dma_start(t[:], P_wproj[ot * 128:(ot + 1) * 128, :])
            wpj.append(t)
        # [head-slot][batch]
        qT = [[persist.tile([128, S], bf16, tag=f"qT{h}{bb}", name=f"qT{h}{bb}")
               for bb in range(B)] for h in range(2)]
        kT = [[persist.tile([128, S], bf16, tag=f"kT{h}{bb}", name=f"kT{h}{bb}")
               for bb in range(B)] for h in range(2)]
        v_sb = [[persist.tile([128, 2 * HD], bf16, tag=f"v{t}{bb}", name=f"v{t}{bb}")
                 for t in range(NT)] for bb in range(B)]
        aT = [persist.tile([128, 512], bf16, tag=f"aT{ot}", name=f"aT{ot}")
              for ot in range(H * HD // 128)]

        with ExitStack() as ctxC:
            pc = ctxC.enter_context(tc.tile_pool(name="pc", bufs=1))
            pc_ps = ctxC.enter_context(
                tc.tile_pool(name="pc_ps", bufs=6, space="PSUM"))

            wdkn_sb = []
            for rt in range(KVR // 128):
                t = pc.tile([128, 2 * ND], bf16, tag=f"wdkn{rt}", name=f"wdkn{rt}")
                nc.sync.dma_start(t[:], P_wdkn[rt * 128:(rt + 1) * 128, :])
                wdkn_sb.append(t)
            wdv_sb = []
            for rt in range(KVR // 128):
                t = pc.tile([128, 2 * HD], bf16, tag=f"wdv{rt}", name=f"wdv{rt}")
                nc.sync.dma_start(t[:], P_wdv[rt * 128:(rt + 1) * 128, :])
                wdv_sb.append(t)

            nkvT = [[], []]
            for bb in range(B):
                for rt in range(KVR // 128):
                    t = pc.tile([128, S], bf16, tag=f"nkvT{bb}{rt}", name=f"nkvT{bb}{rt}")
                    nc.sync.dma_start(
                        t[:].rearrange("p (g c) -> p g c", g=G),
                        gkv_out[4 * bb:4 * bb + 4,
                                rt * 128:(rt + 1) * 128, :].rearrange(
                            "g p c -> p g c"))
                    nkvT[bb].append(t)
            for hl in range(2):
                roff = 64 if hl == 0 else 0   # rope slot: even [nope|rope]
                qoff = 0 if hl == 0 else 64
                koff = 64 if hl == 0 else 0
                for bb in range(B):
                    # shared (already rotated) krope -> qT rope slot
                    nc.sync.dma_start(
                        qT[hl][bb][roff:roff + 64, :].rearrange(
                            "p (g c) -> p g c", g=G),
                        gkv_out[4 * bb:4 * bb + 4, KVR:KVW, :].rearrange(
                            "g p c -> p g c"))
                    # q A2A: nope half -> qT, rotated-rope half -> kT
                    nc.sync.dma_start(
                        qT[hl][bb][qoff:qoff + 64, :].rearrange(
                            "p (g c) -> p g c", g=G),
                        gqa_out[4 * bb:4 * bb + 4,
                                hl * 128:hl * 128 + 64, :].rearrange(
                            "g p c -> p g c"))
                    nc.sync.dma_start(
                        kT[hl][bb][koff:koff + 64, :].rearrange(
                            "p (g c) -> p g c", g=G),
                        gqa_out[4 * bb:4 * bb + 4,
                                hl * 128 + 64:hl * 128 + 128, :].rearrange(
                            "g p c -> p g c"))

            for bb in range(B):
                # ---- v (natural layout) ----
                for st in range(NT):
                    ps = pc_ps.tile([128, 2 * HD], f32, tag="dec_ps", name="dec_ps")
                    for rt in range(KVR // 128):
                        nc.tensor.matmul(
                            ps[:], nkvT[bb][rt][:, st * 128:(st + 1) * 128],
                            wdv_sb[rt][:],
                            start=rt == 0, stop=rt == KVR // 128 - 1)
                    nc.scalar.copy(v_sb[bb][st][:], ps[:])

                # ---- k_nope: head-pair packed, transposed layout ----
                psl = [pc_ps.tile([128, 512], f32, tag="dec_ps", name="dec_ps")
                       for _ in range(S // 512)]
                for rt in range(KVR // 128):
                    stat = wdkn_sb[rt][:]
                    for sc4 in range(S // 512):
                        nc.tensor.matmul(
                            psl[sc4][:], stat,
                            nkvT[bb][rt][:, sc4 * 512:(sc4 + 1) * 512],
                            start=rt == 0, stop=rt == KVR // 128 - 1)
                for sc4 in range(S // 512):
                    sl = slice(sc4 * 512, (sc4 + 1) * 512)
                    # even head: nope at partitions 0:64
                    nc.vector.tensor_copy(kT[0][bb][0:64, sl], psl[sc4][0:64, :])
                    # odd head: half-swapped -> nope at partitions 64:128
                    nc.vector.tensor_copy(kT[1][bb][64:128, sl], psl[sc4][64:128, :])

        # ================= Phase D: causal SDPA (2 heads x 2 batches) ==========
        with ExitStack() as ctxD:
            pd_mv = ctxD.enter_context(tc.tile_pool(name="pd_mv", bufs=6))
            pd_probs = ctxD.enter_context(tc.tile_pool(name="pd_probs", bufs=6))
            pd_sc = ctxD.enter_context(
                tc.tile_pool(name="pd_sc", bufs=4, space="PSUM"))
            pd_acc = ctxD.enter_context(
                tc.tile_pool(name="pd_acc", bufs=2, space="PSUM"))

            for hl in range(2):
                vcol = slice(hl * HD, (hl + 1) * HD)
                for bb in range(B):
                    qTi, kTi = qT[hl][bb], kT[hl][bb]
                    for w in range(NW):
                        nk = 4 * (w + 1)
                        den = pd_acc.tile([128, 512], f32, tag="den", name="den")
                        att = pd_acc.tile([128, 512], f32, tag="att", name="att")
                        for kt in range(nk):
                            off = max(0, 128 * kt - 512 * w)
                            ssc = pd_sc.tile([128, 512], f32,
                                             tag="ssc", name="ssc")
                            nc.tensor.matmul(
                                ssc[:, off:512],
                                kTi[:, kt * 128:(kt + 1) * 128],
                                qTi[:, 512 * w + off:512 * (w + 1)],
                                start=True, stop=True)
                            if kt >= 4 * w:   # block with the diagonal
                                nc.vector.tensor_add(
                                    ssc[:, off:off + 128],
                                    ssc[:, off:off + 128], mask_sb[:])
                            probs = pd_probs.tile([128, 512], bf16,
                                                  tag="probs", name="probs")
                            nc.scalar.activation(
                                probs[:, off:512], ssc[:, off:512],
                                AF.Exp, scale=SCALE)
                            nc.tensor.matmul(
                                den[:, off:512], ones_sb[:], probs[:, off:512],
                                start=kt == 0, stop=kt == nk - 1)
                            nc.tensor.matmul(
                                att[:, off:512], v_sb[bb][kt][:, vcol],
                                probs[:, off:512],
                                start=kt == 0, stop=kt == nk - 1)
                        rec = pd_mv.tile([128, 512], f32, tag="rec", name="rec")
                        nc.vector.reciprocal(rec[:], den[:])
                        outT = pd_mv.tile([128, 512], bf16, tag="outT", name="outT")
                        nc.vector.tensor_mul(outT[:], att[:], rec[:])
                        nc.sync.dma_start(ago_in[hl][4 * bb + w, :, :], outT[:])
                nc.gpsimd.collective_compute(
                    "AllToAll", mybir.AluOpType.bypass,
                    replica_groups=groups,
                    ins=[ago_in[hl].opt()], outs=[ago_out[hl].opt()])
                # prefetch this head-slot's A2A results for the projection
                for i in range(NC):
                    nc.sync.dma_start(aT[2 * i + hl][:], ago_out[hl][i, :, :])

        # ======= S-sharded projection: own 512 rows x full w_proj =======
        with ExitStack() as ctxE:
            pe = ctxE.enter_context(tc.tile_pool(name="pe", bufs=1))
            pe_mv = ctxE.enter_context(tc.tile_pool(name="pe_mv", bufs=4))
            pe_ps = ctxE.enter_context(
                tc.tile_pool(name="pe_ps", bufs=4, space="PSUM"))

            # pass 1: head-slot-0 heads, runs while the hl=1 A2A is in flight
            o_acc = [pe.tile([128, H * HD], f32, tag=f"oacc{st}", name=f"oacc{st}")
                     for st in range(SC // 128)]
            for st in range(SC // 128):
                for half in range(2):
                    psc = [pe_ps.tile([128, 512], f32, tag="proj_ps",
                                      name="proj_ps") for _ in range(2)]
                    for i in range(NC):
                        ot = 2 * i
                        for k2 in range(2):
                            cch = 2 * half + k2
                            nc.tensor.matmul(
                                psc[k2][:], aT[ot][:, st * 128:(st + 1) * 128],
                                wpj[ot][:, cch * 512:(cch + 1) * 512],
                                start=i == 0, stop=i == NC - 1)
                    for k2 in range(2):
                        cch = 2 * half + k2
                        nc.scalar.copy(
                            o_acc[st][:, cch * 512:(cch + 1) * 512], psc[k2][:])
            # pass 2: head-slot-1 heads, add pass-1 partials, write out
            for st in range(SC // 128):
                o_sb = pe_mv.tile([128, H * HD], f32, tag="o_sb", name="o_sb")
                for half in range(2):
                    psc = [pe_ps.tile([128, 512], f32, tag="proj_ps",
                                      name="proj_ps") for _ in range(2)]
                    for i in range(NC):
                        ot = 2 * i + 1
                        for k2 in range(2):
                            cch = 2 * half + k2
                            nc.tensor.matmul(
                                psc[k2][:], aT[ot][:, st * 128:(st + 1) * 128],
                                wpj[ot][:, cch * 512:(cch + 1) * 512],
                                start=i == 0, stop=i == NC - 1)
                    for k2 in range(2):
                        cch = 2 * half + k2
                        nc.vector.tensor_add(
                            o_sb[:, cch * 512:(cch + 1) * 512], psc[k2][:],
                            o_acc[st][:, cch * 512:(cch + 1) * 512])
                nc.sync.dma_start(
                    P_out[st * 128:(st + 1) * 128, :], o_sb[:])

    nc.compile()
    return nc


def _get_nc():
    if "nc" not in _cached:
        _cached["nc"] = _build()
    return _cached["nc"]


def _prep_inputs(inputs):
    x = np.asarray(inputs["x"], np.float32)
    fc = np.asarray(inputs["freqs_cos"], np.float32)   # [S, 32]
    fs = np.asarray(inputs["freqs_sin"], np.float32)
    w_cq = np.asarray(inputs["w_cq"], np.float32)
    w_dq_nope = np.asarray(inputs["w_dq_nope"], np.float32)
    w_dq_rope = np.asarray(inputs["w_dq_rope"], np.float32)
    w_ckv = np.asarray(inputs["w_ckv"], np.float32)
    w_dk_nope = np.asarray(inputs["w_dk_nope"], np.float32)
    w_dv = np.asarray(inputs["w_dv"], np.float32)
    w_krope = np.asarray(inputs["w_krope"], np.float32)
    w_proj = np.asarray(inputs["w_proj"], np.float32)
    qw = np.asarray(inputs["q_norm_w"], np.float32)
    kvw = np.asarray(inputs["kv_norm_w"], np.float32)

    perm = np.concatenate([np.arange(0, RD, 2), np.arange(1, RD, 2)])

    wlat = np.concatenate(
        [w_cq.T, w_ckv.T, w_krope[perm, :].T], axis=1).astype(BF)  # [D, LATW]
    wdqn = (w_dq_nope * qw[None, :])          # [H*ND, QR]
    wdqr = (w_dq_rope * qw[None, :]).reshape(H, RD, QR)[:, perm, :]
    wdkn = (w_dk_nope * kvw[None, :])
    wdv = (w_dv * kvw[None, :])

    wdqn_T = np.ascontiguousarray(wdqn.T).astype(BF)               # [QR, H*ND]
    wdqr_T = np.ascontiguousarray(wdqr.reshape(H * RD, QR).T).astype(BF)
    wprojT = np.ascontiguousarray(w_proj.T).astype(BF)             # [H*HD, H*HD]

    csA = np.concatenate([fc, fs], axis=1).astype(BF)   # [S, 64]
    csB = np.concatenate([fs, fc], axis=1).astype(BF)
    csA8 = np.tile(csA, (1, 8))                          # [S, 512]
    csB8 = np.tile(csB, (1, 8))
    maskT = np.zeros((128, 128), np.float32)
    il, jl = np.tril_indices(128, -1)   # sq < sk  -> masked
    maskT[il, jl] = NEG

    in_maps = []
    for c in range(NC):
        b, g = divmod(c, G)
        hsl = slice(2 * c, 2 * c + 2)     # SDPA heads owned by this core
        xT_c = np.ascontiguousarray(x[b].T[:, g * SC:(g + 1) * SC]).astype(BF)
        wdkn_c = np.ascontiguousarray(
            wdkn.reshape(H, ND, KVR)[hsl].reshape(2 * ND, KVR).T).astype(BF)
        wdv_c = np.ascontiguousarray(
            wdv.reshape(H, HD, KVR)[hsl].reshape(2 * HD, KVR).T).astype(BF)
        ssl = slice(g * SC, (g + 1) * SC)
        in_maps.append({
            "xT": xT_c,
            "wlat": wlat,
            "wdqn": wdqn_T,
            "wdqr": wdqr_T,
            "wdkn": wdkn_c,
            "wdv": wdv_c,
            "wproj": wprojT,
            "csAc": np.ascontiguousarray(csA[ssl]),
            "csBc": np.ascontiguousarray(csB[ssl]),
            "csAc8": np.ascontiguousarray(csA8[ssl]),
            "csBc8": np.ascontiguousarray(csB8[ssl]),
            "maskT": maskT,
        })
    return in_maps


def _assemble(results):
    out = np.zeros((B, S, H * HD), np.float32)
    for c in range(NC):
        b, g = divmod(c, G)
        out[b, g * SC:(g + 1) * SC, :] = results[c]["out"]
    return out


def kernel(**inputs) -> np.ndarray:
    from concourse.bass_utils import run_bass_kernel_spmd
    nc = _get_nc()
    in_maps = _prep_inputs(inputs)
    res = run_bass_kernel_spmd(nc, in_maps, core_ids=list(range(NC)))
    return _assemble(res.results)


# revision 56
# speedup vs baseline: 1.0214x; 1.0214x over previous
"""Multi-Head Latent Attention (MLA) Trainium2 kernel, 8-core SPMD.

Latent sharding: core c -> batch b = c // 4, S-chunk g = c % 4 (512 rows).
SDPA/decompress sharding: core c -> global heads {2c, 2c+1} for BOTH
batches (4 head-instances per core) so that every collective is a single
8-wide op on [[0..7]] (4-wide AllToAll is unsupported on trn2 mesh).

Flow:
 - Phase A: latent path (cq / ckv / k_rope_raw + rmsnorm) on own chunk;
   kv latent (+rotated krope) AllGathered 8-wide early.
 - Phase B: decompress q_nope and (rotated) q_rope for ALL 16 heads on own
   chunk (same total FLOPs as 4-head x full-S), then ONE 8-wide AllToAll
   delivers each head-pair to its owner.  No serial AG->decompress stall.
 - Phase C: decompress k/v for own 2 heads x both batches from gathered
   latent; assemble qT/kT.
 - Phase D: causal SDPA per (head, batch), scores transposed ([sk, sq]);
   denominators via ones-matmul.  Attention outputs leave through one
   8-wide AllToAll per head-slot (block = (batch, sq-window)), pipelined.
 - Phase E: after the A2As each core holds all 16 heads restricted to its
   own (batch, 512-row chunk); projection is S-sharded with full w_proj.

Layout notes:
 - All matmul operands bf16 (fp32 PSUM accumulate); rope pair-dims are
   permuted (even dims first) so rotation works on contiguous 32-blocks,
   applied identically to q and k so dot products are unchanged.
 - Odd heads use a half-swapped partition layout ([rope | nope]) in both
   qT and kT so every PSUM eviction is partition-aligned.
 - q_norm_w / kv_norm_w are folded into the decompress weights on host.
 - q A2A block j (256 rows): per head slot (2): [0:64] q_nope (-> qT),
   [64:128] rotated per-head q_rope (-> kT rope slot, per the reference's
   rope swap).
"""

import sys

for _p in ("/opt/trn_rl_repo", "/opt/pypackages"):
    if _p not in sys.path:
        sys.path.append(_p)

import numpy as np
import ml_dtypes

B, S, D = 2, 2048, 2048
H, HD, RD, ND = 16, 128, 64, 64
QR, KVR = 1536, 512
EPS = 1e-6
G = 4            # S-chunks per batch
NC = 8
SC = S // G      # latent-path S chunk per core
NT = S // 128    # 16 s-tiles
NW = S // 512    # 4  sq windows
LATW = QR + KVR + RD   # 2112 = packed cq|ckv|krope width
SCALE = 1.0 / float(np.sqrt(HD))
NEG = -30000.0   # additive mask; * SCALE stays << exp underflow

BF = ml_dtypes.bfloat16

_cached = {}


def _build():
    import concourse.bass as bass
    import concourse.mybir as mybir
    import concourse.tile as tile
    from concourse import bacc
    from concourse.masks import make_identity
    from contextlib import ExitStack

    f32 = mybir.dt.float32
    bf16 = mybir.dt.bfloat16
    f8 = mybir.dt.float8e4
    DR = mybir.MatmulPerfMode.DoubleRow
    AF = mybir.ActivationFunctionType

    nc = bacc.Bacc()

    # ---- parameters (per-core host-prepped) ----
    P_xT = nc.declare_dram_parameter("xT", [D, SC], bf16, isOutput=False)
    P_wlat = nc.declare_dram_parameter("wlat", [D, LATW], bf16, isOutput=False)
    P_wdqn = nc.declare_dram_parameter("wdqn", [QR, H * ND], bf16, isOutput=False)
    P_wdqr = nc.declare_dram_parameter("wdqr", [QR, H * RD], bf16, isOutput=False)
    P_wdkn = nc.declare_dram_parameter("wdkn", [KVR, 2 * ND], bf16, isOutput=False)
    P_wdv = nc.declare_dram_parameter("wdv", [KVR, 2 * HD], bf16, isOutput=False)
    P_wproj = nc.declare_dram_parameter("wproj", [H * HD, H * HD], bf16, isOutput=False)
    # rope tables (own chunk), A variant = [cos|sin], B = [sin|cos]
    P_csAc = nc.declare_dram_parameter("csAc", [SC, RD], bf16, isOutput=False)
    P_csBc = nc.declare_dram_parameter("csBc", [SC, RD], bf16, isOutput=False)
    P_csAc8 = nc.declare_dram_parameter("csAc8", [SC, 8 * RD], bf16, isOutput=False)
    P_csBc8 = nc.declare_dram_parameter("csBc8", [SC, 8 * RD], bf16, isOutput=False)
    P_mask = nc.declare_dram_parameter("maskT", [128, 128], f32, isOutput=False)
    P_out = nc.declare_dram_parameter("out", [SC, H * HD], f32, isOutput=True)

    groups = [[0, 1, 2, 3, 4, 5, 6, 7]]

    with ExitStack() as top:
        tc = top.enter_context(tile.TileContext(nc))

        dram = top.enter_context(tc.tile_pool(name="dram", bufs=1, space="DRAM"))
        KVW = KVR + RD   # 576
        gkv_in = dram.tile([KVW, SC], bf16, tag="gkv_in", name="gkv_in")
        gkv_out = dram.tile([NC, KVW, SC], bf16, tag="gkv_out", name="gkv_out")
        # q A2A: block j = heads {2j, 2j+1} for my S chunk; 128 rows/head
        gqa_in = dram.tile([NC, 256, SC], bf16, tag="gqa_in", name="gqa_in")
        gqa_out = dram.tile([NC, 256, SC], bf16, tag="gqa_out", name="gqa_out")
        # attn-out A2A per local head slot: block j = (batch j//4, window j%4)
        ago_in = [dram.tile([NC, HD, 512], bf16, tag=f"ago_in{h}", name=f"ago_in{h}")
                  for h in range(2)]
        ago_out = [dram.tile([NC, HD, 512], bf16, tag=f"ago_out{h}", name=f"ago_out{h}")
                   for h in range(2)]

        const = top.enter_context(tc.tile_pool(name="const", bufs=1))
        ident = const.tile([128, 128], bf16, tag="ident", name="ident")
        make_identity(nc, ident)
        ones_sb = const.tile([128, 128], bf16, tag="ones", name="ones")
        nc.vector.memset(ones_sb[:], 1.0)
        mask_sb = const.tile([128, 128], f32, tag="mask", name="mask")
        nc.sync.dma_start(mask_sb[:], P_mask[:])
        eps_sb = const.tile([128, 1], f32, tag="eps", name="eps")
        nc.vector.memset(eps_sb[:], EPS)

        with ExitStack() as ctxAB:
            # transposed normed q latent, persists from Phase A into Phase B
            nqp = ctxAB.enter_context(tc.tile_pool(name="nqp", bufs=1))
            nqTl = [nqp.tile([128, SC], bf16, tag=f"nqT{rt}", name=f"nqT{rt}")
                    for rt in range(QR // 128)]
            # q decompress weights + tables (used inside pass 2); DMAs are
            # issued after pass 1 so they don't crowd pass 1's inputs
            wdqr_sb = [nqp.tile([128, H * RD], bf16, tag=f"wdqr{rt}",
                                name=f"wdqr{rt}") for rt in range(QR // 128)]
            wdqn_sb = [nqp.tile([128, H * ND], bf16, tag=f"wdqn{rt}",
                                name=f"wdqn{rt}") for rt in range(QR // 128)]
            csA8_sb = [nqp.tile([128, 512], bf16, tag=f"csA8{st}",
                                name=f"csA8{st}") for st in range(SC // 128)]
            csB8_sb = [nqp.tile([128, 512], bf16, tag=f"csB8{st}",
                                name=f"csB8{st}") for st in range(SC // 128)]

            # ================= Phase A: latent path on own S chunk =============
            with ExitStack() as ctxA:
                pa = ctxA.enter_context(tc.tile_pool(name="pa", bufs=1))
                pa_mv = ctxA.enter_context(tc.tile_pool(name="pa_mv", bufs=3))
                pa_ps = ctxA.enter_context(
                    tc.tile_pool(name="pa_ps", bufs=6, space="PSUM"))
                pa_tp = ctxA.enter_context(
                    tc.tile_pool(name="pa_tp", bufs=2, space="PSUM"))

                xT_sb = []
                wlat_sb = []
                for dt_ in range(D // 128):
                    xt = pa.tile([128, SC], bf16, tag=f"xT{dt_}", name=f"xT{dt_}")
                    nc.sync.dma_start(xt[:], P_xT[dt_ * 128:(dt_ + 1) * 128, :])
                    xT_sb.append(xt)
                    wl = pa.tile([128, LATW], bf16, tag=f"wlat{dt_}", name=f"wlat{dt_}")
                    nc.sync.dma_start(wl[:, 1536:LATW],
                                      P_wlat[dt_ * 128:(dt_ + 1) * 128, 1536:LATW])
                    wlat_sb.append(wl)
                for dt_ in range(D // 128):
                    nc.sync.dma_start(wlat_sb[dt_][:, 0:1536],
                                      P_wlat[dt_ * 128:(dt_ + 1) * 128, 0:1536])
                csAc_sb, csBc_sb = [], []
                for st in range(SC // 128):
                    t = pa.tile([128, RD], bf16, tag=f"csAc{st}", name=f"csAc{st}")
                    nc.sync.dma_start(t[:], P_csAc[st * 128:(st + 1) * 128, :])
                    csAc_sb.append(t)
                    t = pa.tile([128, RD], bf16, tag=f"csBc{st}", name=f"csBc{st}")
                    nc.sync.dma_start(t[:], P_csBc[st * 128:(st + 1) * 128, :])
                    csBc_sb.append(t)

                # ---- PASS 1: kv + krope columns only, so their AllGather
                # fires early and absorbs cross-rank skew ----
                for st in range(SC // 128):
                    pkv = pa_ps.tile([128, 512], f32, tag="lat_ps", name="lat_ps")
                    pkr = pa_ps.tile([128, RD], f32, tag="lat_ps", name="lat_ps")
                    for dt_ in range(D // 128):
                        stat = xT_sb[dt_][:, st * 128:(st + 1) * 128]
                        first, last = dt_ == 0, dt_ == D // 128 - 1
                        nc.tensor.matmul(
                            pkv[:], stat, wlat_sb[dt_][:, 1536:2048],
                            start=first, stop=last)
                        nc.tensor.matmul(
                            pkr[:], stat, wlat_sb[dt_][:, 2048:LATW],
                            start=first, stop=last)
                    kvn_sb = pa_mv.tile([128, KVW], bf16, tag="kvn_sb", name="kvn_sb")
                    acckv = pa_mv.tile([128, 1], f32, tag="acckv", name="acckv")
                    sqkv = pa_mv.tile([128, 512], f32, tag="sqkv", name="sqkv")
                    nc.scalar.activation(sqkv[:], pkv[:], AF.Square,
                                         accum_out=acckv[:])
                    stdkv = pa_mv.tile([128, 1], f32, tag="stdkv", name="stdkv")
                    nc.scalar.activation(stdkv[:], acckv[:], AF.Sqrt,
                                         bias=eps_sb[:], scale=1.0 / KVR)
                    rkv = pa_mv.tile([128, 1], f32, tag="rkv", name="rkv")
                    nc.vector.reciprocal(rkv[:], stdkv[:])
                    nc.vector.tensor_scalar_mul(kvn_sb[:, 0:512], pkv[:], rkv[:])
                    # krope: rotate (no norm)
                    kr_raw = pa_mv.tile([128, RD], bf16, tag="kr_raw", name="kr_raw")
                    nc.scalar.copy(kr_raw[:], pkr[:])
                    pr1 = pa_mv.tile([128, RD], bf16, tag="pr1", name="pr1")
                    pr2 = pa_mv.tile([128, RD], bf16, tag="pr2", name="pr2")
                    nc.vector.tensor_mul(pr1[:], kr_raw[:], csAc_sb[st][:])
                    nc.vector.tensor_mul(pr2[:], kr_raw[:], csBc_sb[st][:])
                    nc.vector.tensor_sub(kvn_sb[:, 512:544],
                                         pr1[:, 0:32], pr1[:, 32:64])
                    nc.vector.tensor_add(kvn_sb[:, 544:576],
                                         pr2[:, 0:32], pr2[:, 32:64])
                    for rt in range(4):
                        tp = pa_tp.tile([128, 128], bf16, tag="tp", name="tp")
                        nc.tensor.transpose(
                            tp[:], kvn_sb[:, rt * 128:(rt + 1) * 128], ident[:])
                        tps = pa_mv.tile([128, 128], bf16, tag="tps", name="tps")
                        nc.scalar.copy(tps[:], tp[:])
                        nc.scalar.dma_start(
                            gkv_in[rt * 128:(rt + 1) * 128,
                                   st * 128:(st + 1) * 128], tps[:])
                    tp = pa_tp.tile([128, 128], bf16, tag="tp", name="tp")
                    nc.tensor.transpose(tp[0:64, :], kvn_sb[:, 512:576], ident[:])
                    tps = pa_mv.tile([128, 128], bf16, tag="tps", name="tps")
                    nc.scalar.copy(tps[0:64, :], tp[0:64, :])
                    nc.scalar.dma_start(
                        gkv_in[KVR:KVW, st * 128:(st + 1) * 128], tps[0:64, :])

                nc.gpsimd.collective_compute(
                    "AllGather", mybir.AluOpType.bypass,
                    replica_groups=groups,
                    ins=[gkv_in.opt()], outs=[gkv_out.opt()])

                for rt in range(QR // 128):
                    nc.sync.dma_start(wdqr_sb[rt][:],
                                      P_wdqr[rt * 128:(rt + 1) * 128, :])
                for st in range(SC // 128):
                    nc.sync.dma_start(csA8_sb[st][:],
                                      P_csAc8[st * 128:(st + 1) * 128, :])
                    nc.sync.dma_start(csB8_sb[st][:],
                                      P_csBc8[st * 128:(st + 1) * 128, :])
                for rt in range(QR // 128):
                    nc.sync.dma_start(wdqn_sb[rt][:],
                                      P_wdqn[rt * 128:(rt + 1) * 128, :])

                # ---- PASS 2: q columns (cq + rmsnorm), transposed into SBUF ----
                for st in range(SC // 128):
                    ps = []
                    for j in range(3):
                        p = pa_ps.tile([128, 512], f32, tag="lat_ps", name="lat_ps")
                        ps.append(p)
                    for dt_ in range(D // 128):
                        stat = xT_sb[dt_][:, st * 128:(st + 1) * 128]
                        first, last = dt_ == 0, dt_ == D // 128 - 1
                        for j in range(3):
                            nc.tensor.matmul(
                                ps[j][:], stat,
                                wlat_sb[dt_][:, j * 512:(j + 1) * 512],
                                start=first, stop=last)
                    norm_sb = pa_mv.tile([128, QR], bf16, tag="norm_sb", name="norm_sb")
                    acc = [pa_mv.tile([128, 1], f32, tag=f"acc{i}", name=f"acc{i}")
                           for i in range(3)]
                    for i in range(3):
                        sq = pa_mv.tile([128, 512], f32, tag=f"sq{i}", name=f"sq{i}")
                        nc.scalar.activation(sq[:], ps[i][:], AF.Square,
                                             accum_out=acc[i][:])
                    accq = pa_mv.tile([128, 1], f32, tag="accq", name="accq")
                    nc.vector.tensor_add(accq[:], acc[0][:], acc[1][:])
                    nc.vector.tensor_add(accq[:], accq[:], acc[2][:])
                    stdq = pa_mv.tile([128, 1], f32, tag="stdq", name="stdq")
                    nc.scalar.activation(stdq[:], accq[:], AF.Sqrt,
                                         bias=eps_sb[:], scale=1.0 / QR)
                    rq = pa_mv.tile([128, 1], f32, tag="rq", name="rq")
                    nc.vector.reciprocal(rq[:], stdq[:])
                    for j in range(3):
                        nc.vector.tensor_scalar_mul(
                            norm_sb[:, j * 512:(j + 1) * 512], ps[j][:], rq[:])
                    for rt in range(12):
                        tp = pa_tp.tile([128, 128], bf16, tag="tp", name="tp")
                        nc.tensor.transpose(
                            tp[:], norm_sb[:, rt * 128:(rt + 1) * 128], ident[:])
                        nc.scalar.copy(
                            nqTl[rt][:, st * 128:(st + 1) * 128], tp[:])

                    # ---- q_rope for this st: decompress (natural), rotate,
                    # transpose into A2A slots -- pipelined behind pass 2 ----
                    pn = [pa_ps.tile([128, 512], f32, tag="lat_ps", name="lat_ps")
                          for _ in range(2)]
                    for rt in range(QR // 128):
                        stat = nqTl[rt][:, st * 128:(st + 1) * 128]
                        for j in range(2):
                            nc.tensor.matmul(
                                pn[j][:], stat,
                                wdqr_sb[rt][:, j * 512:(j + 1) * 512],
                                start=rt == 0, stop=rt == QR // 128 - 1)
                    for j in range(2):
                        pr1 = pa_mv.tile([128, 512], bf16, tag="qpr1", name="qpr1")
                        pr2 = pa_mv.tile([128, 512], bf16, tag="qpr2", name="qpr2")
                        nc.vector.tensor_mul(pr1[:], pn[j][:], csA8_sb[st][:])
                        nc.vector.tensor_mul(pr2[:], pn[j][:], csB8_sb[st][:])
                        rot = pa_mv.tile([128, 512], bf16, tag="qrot", name="qrot")
                        r3a = rot[:].rearrange("p (h two f) -> p h two f", two=2, f=32)
                        p3a = pr1[:].rearrange("p (h two f) -> p h two f", two=2, f=32)
                        p3b = pr2[:].rearrange("p (h two f) -> p h two f", two=2, f=32)
                        nc.vector.tensor_sub(r3a[:, :, 0, :], p3a[:, :, 0, :],
                                             p3a[:, :, 1, :])
                        nc.vector.tensor_add(r3a[:, :, 1, :], p3b[:, :, 0, :],
                                             p3b[:, :, 1, :])
                        for pr in range(4):   # head pair within this 8-head tile
                            blk = 4 * j + pr
                            tp = pa_tp.tile([128, 128], bf16, tag="tp", name="tp")
                            nc.tensor.transpose(
                                tp[:], rot[:, pr * 128:(pr + 1) * 128], ident[:])
                            tps = pa_mv.tile([128, 128], bf16, tag="qtps", name="qtps")
                            nc.scalar.copy(tps[:], tp[:])
                            eng = (nc.scalar, nc.gpsimd, nc.scalar, nc.gpsimd)[pr]
                            eng.dma_start(
                                gqa_in[blk, 64:128, st * 128:(st + 1) * 128],
                                tps[0:64, :])
                            eng.dma_start(
                                gqa_in[blk, 192:256, st * 128:(st + 1) * 128],
                                tps[64:128, :])

            # ======= Phase B: q_nope decompress (transposed form) =======
            with ExitStack() as ctxB:
                pb_mv = ctxB.enter_context(tc.tile_pool(name="pb_mv", bufs=4))
                pb_ps = ctxB.enter_context(
                    tc.tile_pool(name="pb_ps", bufs=3, space="PSUM"))
                # psum tile p = A2A block p (heads 2p, 2p+1)
                for p in range(H // 2):
                    ps = pb_ps.tile([128, 512], f32, tag="qn_ps", name="qn_ps")
                    for rt in range(QR // 128):
                        nc.tensor.matmul(
                            ps[:], wdqn_sb[rt][:, p * 128:(p + 1) * 128],
                            nqTl[rt][:],
                            start=rt == 0, stop=rt == QR // 128 - 1)
                    qn_sb = pb_mv.tile([128, 512], bf16, tag="qn_sb", name="qn_sb")
                    nc.scalar.copy(qn_sb[:], ps[:])
                    nc.scalar.dma_start(gqa_in[p, 0:64, :], qn_sb[0:64, :])
                    nc.scalar.dma_start(gqa_in[p, 128:192, :], qn_sb[64:128, :])

            nc.gpsimd.collective_compute(
                "AllToAll", mybir.AluOpType.bypass,
                replica_groups=groups,
                ins=[gqa_in.opt()], outs=[gqa_out.opt()])

        # ================= Phase C: decompress k/v, assemble q =================
        persist = top.enter_context(tc.tile_pool(name="persist", bufs=1))
        wpj = []
        for ot in range(H * HD // 128):
            t = persist.tile([128, H * HD], bf16, tag=f"wpj{ot}", name=f"wpj{ot}")
            nc.sync.dma_start(t[:], P_wproj[ot * 128:(ot + 1) * 128, :])
            wpj.append(t)
        # [head-slot][batch]
        qT = [[persist.tile([128, S], bf16, tag=f"qT{h}{bb}", name=f"qT{h}{bb}")
               for bb in range(B)] for h in range(2)]
        kT = [[persist.tile([128, S], bf16, tag=f"kT{h}{bb}", name=f"kT{h}{bb}")
               for bb in range(B)] for h in range(2)]
        v_sb = [[persist.tile([128, 2 * HD], bf16, tag=f"v{t}{bb}", name=f"v{t}{bb}")
                 for t in range(NT)] for bb in range(B)]
        aT = [persist.tile([128, 512], bf16, tag=f"aT{ot}", name=f"aT{ot}")
              for ot in range(H * HD // 128)]

        with ExitStack() as ctxC:
            pc = ctxC.enter_context(tc.tile_pool(name="pc", bufs=1))
            pc_ps = ctxC.enter_context(
                tc.tile_pool(name="pc_ps", bufs=4, space="PSUM"))

            wdkn_sb = []
            for rt in range(KVR // 128):
                t = pc.tile([128, 2 * ND], bf16, tag=f"wdkn{rt}", name=f"wdkn{rt}")
                nc.sync.dma_start(t[:], P_wdkn[rt * 128:(rt + 1) * 128, :])
                wdkn_sb.append(t)
            wdv_sb = []
            for rt in range(KVR // 128):
                t = pc.tile([128, 2 * HD], bf16, tag=f"wdv{rt}", name=f"wdv{rt}")
                nc.sync.dma_start(t[:], P_wdv[rt * 128:(rt + 1) * 128, :])
                wdv_sb.append(t)

            nkvT = [[], []]
            for bb in range(B):
                for rt in range(KVR // 128):
                    t = pc.tile([128, S], bf16, tag=f"nkvT{bb}{rt}", name=f"nkvT{bb}{rt}")
                    nc.sync.dma_start(
                        t[:].rearrange("p (g c) -> p g c", g=G),
                        gkv_out[4 * bb:4 * bb + 4,
                                rt * 128:(rt + 1) * 128, :].rearrange(
                            "g p c -> p g c"))
                    nkvT[bb].append(t)
            for hl in range(2):
                roff = 64 if hl == 0 else 0   # rope slot: even [nope|rope]
                qoff = 0 if hl == 0 else 64
                koff = 64 if hl == 0 else 0
                for bb in range(B):
                    # shared (already rotated) krope -> qT rope slot
                    nc.sync.dma_start(
                        qT[hl][bb][roff:roff + 64, :].rearrange(
                            "p (g c) -> p g c", g=G),
                        gkv_out[4 * bb:4 * bb + 4, KVR:KVW, :].rearrange(
                            "g p c -> p g c"))
                    # q A2A: nope half -> qT, rotated-rope half -> kT
                    nc.sync.dma_start(
                        qT[hl][bb][qoff:qoff + 64, :].rearrange(
                            "p (g c) -> p g c", g=G),
                        gqa_out[4 * bb:4 * bb + 4,
                                hl * 128:hl * 128 + 64, :].rearrange(
                            "g p c -> p g c"))
                    nc.sync.dma_start(
                        kT[hl][bb][koff:koff + 64, :].rearrange(
                            "p (g c) -> p g c", g=G),
                        gqa_out[4 * bb:4 * bb + 4,
                                hl * 128 + 64:hl * 128 + 128, :].rearrange(
                            "g p c -> p g c"))

            for bb in range(B):
                # ---- v (natural layout) ----
                for st in range(NT):
                    ps = pc_ps.tile([128, 2 * HD], f32, tag="dec_ps", name="dec_ps")
                    for rt in range(KVR // 128):
                        nc.tensor.matmul(
                            ps[:], nkvT[bb][rt][:, st * 128:(st + 1) * 128],
                            wdv_sb[rt][:],
                            start=rt == 0, stop=rt == KVR // 128 - 1)
                    nc.scalar.copy(v_sb[bb][st][:], ps[:])

                # ---- k_nope: head-pair packed, transposed layout ----
                psl = [pc_ps.tile([128, 512], f32, tag="dec_ps", name="dec_ps")
                       for _ in range(S // 512)]
                for rt in range(KVR // 128):
                    stat = wdkn_sb[rt][:]
                    for sc4 in range(S // 512):
                        nc.tensor.matmul(
                            psl[sc4][:], stat,
                            nkvT[bb][rt][:, sc4 * 512:(sc4 + 1) * 512],
                            start=rt == 0, stop=rt == KVR // 128 - 1)
                for sc4 in range(S // 512):
                    sl = slice(sc4 * 512, (sc4 + 1) * 512)
                    # even head: nope at partitions 0:64
                    nc.vector.tensor_copy(kT[0][bb][0:64, sl], psl[sc4][0:64, :])
                    # odd head: half-swapped -> nope at partitions 64:128
                    nc.vector.tensor_copy(kT[1][bb][64:128, sl], psl[sc4][64:128, :])

        # ================= Phase D: causal SDPA (2 heads x 2 batches) ==========
        with ExitStack() as ctxD:
            pd_mv = ctxD.enter_context(tc.tile_pool(name="pd_mv", bufs=4))
            pd_probs = ctxD.enter_context(tc.tile_pool(name="pd_probs", bufs=6))
            pd_sc = ctxD.enter_context(
                tc.tile_pool(name="pd_sc", bufs=4, space="PSUM"))
            pd_acc = ctxD.enter_context(
                tc.tile_pool(name="pd_acc", bufs=2, space="PSUM"))

            for hl in range(2):
                vcol = slice(hl * HD, (hl + 1) * HD)
                for bb in range(B):
                    qTi, kTi = qT[hl][bb], kT[hl][bb]
                    for w in range(NW):
                        nk = 4 * (w + 1)
                        den = pd_acc.tile([128, 512], f32, tag="den", name="den")
                        att = pd_acc.tile([128, 512], f32, tag="att", name="att")
                        for kt in range(nk):
                            off = max(0, 128 * kt - 512 * w)
                            ssc = pd_sc.tile([128, 512], f32,
                                             tag="ssc", name="ssc")
                            nc.tensor.matmul(
                                ssc[:, off:512],
                                kTi[:, kt * 128:(kt + 1) * 128],
                                qTi[:, 512 * w + off:512 * (w + 1)],
                                start=True, stop=True)
                            if kt >= 4 * w:   # block with the diagonal
                                nc.vector.tensor_add(
                                    ssc[:, off:off + 128],
                                    ssc[:, off:off + 128], mask_sb[:])
                            probs = pd_probs.tile([128, 512], bf16,
                                                  tag="probs", name="probs")
                            nc.scalar.activation(
                                probs[:, off:512], ssc[:, off:512],
                                AF.Exp, scale=SCALE)
                            nc.tensor.matmul(
                                den[:, off:512], ones_sb[:], probs[:, off:512],
                                start=kt == 0, stop=kt == nk - 1)
                            nc.tensor.matmul(
                                att[:, off:512], v_sb[bb][kt][:, vcol],
                                probs[:, off:512],
                                start=kt == 0, stop=kt == nk - 1)
                        rec = pd_mv.tile([128, 512], f32, tag="rec", name="rec")
                        nc.vector.reciprocal(rec[:], den[:])
                        outT = pd_mv.tile([128, 512], bf16, tag="outT", name="outT")
                        nc.vector.tensor_mul(outT[:], att[:], rec[:])
                        nc.sync.dma_start(ago_in[hl][4 * bb + w, :, :], outT[:])
                nc.gpsimd.collective_compute(
                    "AllToAll", mybir.AluOpType.bypass,
                    replica_groups=groups,
                    ins=[ago_in[hl].opt()], outs=[ago_out[hl].opt()])
                # prefetch this head-slot's A2A results for the projection
                for i in range(NC):
                    nc.sync.dma_start(aT[2 * i + hl][:], ago_out[hl][i, :, :])

        # ======= S-sharded projection: own 512 rows x full w_proj =======
        with ExitStack() as ctxE:
            pe = ctxE.enter_context(tc.tile_pool(name="pe", bufs=1))
            pe_mv = ctxE.enter_context(tc.tile_pool(name="pe_mv", bufs=4))
            pe_ps = ctxE.enter_context(
                tc.tile_pool(name="pe_ps", bufs=4, space="PSUM"))

            # pass 1: head-slot-0 heads, runs while the hl=1 A2A is in flight
            o_acc = [pe.tile([128, H * HD], f32, tag=f"oacc{st}", name=f"oacc{st}")
                     for st in range(SC // 128)]
            for st in range(SC // 128):
                for half in range(2):
                    psc = [pe_ps.tile([128, 512], f32, tag="proj_ps",
                                      name="proj_ps") for _ in range(2)]
                    for i in range(NC):
                        ot = 2 * i
                        for k2 in range(2):
                            cch = 2 * half + k2
                            nc.tensor.matmul(
                                psc[k2][:], aT[ot][:, st * 128:(st + 1) * 128],
                                wpj[ot][:, cch * 512:(cch + 1) * 512],
                                start=i == 0, stop=i == NC - 1)
                    for k2 in range(2):
                        cch = 2 * half + k2
                        nc.scalar.copy(
                            o_acc[st][:, cch * 512:(cch + 1) * 512], psc[k2][:])
            # pass 2: head-slot-1 heads, add pass-1 partials, write out
            for st in range(SC // 128):
                o_sb = pe_mv.tile([128, H * HD], f32, tag="o_sb", name="o_sb")
                for half in range(2):
                    psc = [pe_ps.tile([128, 512], f32, tag="proj_ps",
                                      name="proj_ps") for _ in range(2)]
                    for i in range(NC):
                        ot = 2 * i + 1
                        for k2 in range(2):
                            cch = 2 * half + k2
                            nc.tensor.matmul(
                                psc[k2][:], aT[ot][:, st * 128:(st + 1) * 128],
                                wpj[ot][:, cch * 512:(cch + 1) * 512],
                                start=i == 0, stop=i == NC - 1)
                    for k2 in range(2):
                        cch = 2 * half + k2
                        nc.vector.tensor_add(
                            o_sb[:, cch * 512:(cch + 1) * 512], psc[k2][:],
                            o_acc[st][:, cch * 512:(cch + 1) * 512])
                nc.sync.dma_start(
                    P_out[st * 128:(st + 1) * 128, :], o_sb[:])

    nc.compile()
    return nc


def _get_nc():
    if "nc" not in _cached:
        _cached["nc"] = _build()
    return _cached["nc"]


def _prep_inputs(inputs):
    x = np.asarray(inputs["x"], np.float32)
    fc = np.asarray(inputs["freqs_cos"], np.float32)   # [S, 32]
    fs = np.asarray(inputs["freqs_sin"], np.float32)
    w_cq = np.asarray(inputs["w_cq"], np.float32)
    w_dq_nope = np.asarray(inputs["w_dq_nope"], np.float32)
    w_dq_rope = np.asarray(inputs["w_dq_rope"], np.float32)
    w_ckv = np.asarray(inputs["w_ckv"], np.float32)
    w_dk_nope = np.asarray(inputs["w_dk_nope"], np.float32)
    w_dv = np.asarray(inputs["w_dv"], np.float32)
    w_krope = np.asarray(inputs["w_krope"], np.float32)
    w_proj = np.asarray(inputs["w_proj"], np.float32)
    qw = np.asarray(inputs["q_norm_w"], np.float32)
    kvw = np.asarray(inputs["kv_norm_w"], np.float32)

    perm = np.concatenate([np.arange(0, RD, 2), np.arange(1, RD, 2)])

    wlat = np.concatenate(
        [w_cq.T, w_ckv.T, w_krope[perm, :].T], axis=1).astype(BF)  # [D, LATW]
    wdqn = (w_dq_nope * qw[None, :])          # [H*ND, QR]
    wdqr = (w_dq_rope * qw[None, :]).reshape(H, RD, QR)[:, perm, :]
    wdkn = (w_dk_nope * kvw[None, :])
    wdv = (w_dv * kvw[None, :])

    wdqn_T = np.ascontiguousarray(wdqn.T).astype(BF)               # [QR, H*ND]
    wdqr_T = np.ascontiguousarray(wdqr.reshape(H * RD, QR).T).astype(BF)
    wprojT = np.ascontiguousarray(w_proj.T).astype(BF)             # [H*HD, H*HD]

    csA = np.concatenate([fc, fs], axis=1).astype(BF)   # [S, 64]
    csB = np.concatenate([fs, fc], axis=1).astype(BF)
    csA8 = np.tile(csA, (1, 8))                          # [S, 512]
    csB8 = np.tile(csB, (1, 8))
    maskT = np.zeros((128, 128), np.float32)
    il, jl = np.tril_indices(128, -1)   # sq < sk  -> masked
    maskT[il, jl] = NEG

    in_maps = []
    for c in range(NC):
        b, g = divmod(c, G)
        hsl = slice(2 * c, 2 * c + 2)     # SDPA heads owned by this core
        xT_c = np.ascontiguousarray(x[b].T[:, g * SC:(g + 1) * SC]).astype(BF)
        wdkn_c = np.ascontiguousarray(
            wdkn.reshape(H, ND, KVR)[hsl].reshape(2 * ND, KVR).T).astype(BF)
        wdv_c = np.ascontiguousarray(
            wdv.reshape(H, HD, KVR)[hsl].reshape(2 * HD, KVR).T).astype(BF)
        ssl = slice(g * SC, (g + 1) * SC)
        in_maps.append({
            "xT": xT_c,
            "wlat": wlat,
            "wdqn": wdqn_T,
            "wdqr": wdqr_T,
            "wdkn": wdkn_c,
            "wdv": wdv_c,
            "wproj": wprojT,
            "csAc": np.ascontiguousarray(csA[ssl]),
            "csBc": np.ascontiguousarray(csB[ssl]),
            "csAc8": np.ascontiguousarray(csA8[ssl]),
            "csBc8": np.ascontiguousarray(csB8[ssl]),
            "maskT": maskT,
        })
    return in_maps


def _assemble(results):
    out = np.zeros((B, S, H * HD), np.float32)
    for c in range(NC):
        b, g = divmod(c, G)
        out[b, g * SC:(g + 1) * SC, :] = results[c]["out"]
    return out


def kernel(**inputs) -> np.ndarray:
    from concourse.bass_utils import run_bass_kernel_spmd
    nc = _get_nc()
    in_maps = _prep_inputs(inputs)
    res = run_bass_kernel_spmd(nc, in_maps, core_ids=list(range(NC)))
    return _assemble(res.results)
